# revision 60
# baseline (speedup 1.0000x reference)
"""Trainium2 Bass kernel for GQA attention prefill (Mistral-style, RoPE, causal).

B=1, S=2048, DIM=4096, 32 Q heads / 8 KV heads, HD=128, rope theta 1e6.

Sharding: tensor-parallel over heads across 8 cores. Core i gets Q heads
4i..4i+3 and KV head i. x is replicated (pre-transposed + bf16-cast on host).
Each core computes its 4 heads' attention and a partial output projection
(contraction over its 512 input dims of wo); the host sums the 8 partials
(partials are written bf16 to halve the output DMA).

Per-core dataflow (all matmuls bf16 with fp32 PSUM accumulation):
  phase A (per 512-col s chunk):
    xT tiles [c,s] (lhsT) x wT [c, q|k|v] (rhs) -> psum [d, s]
    (already transposed for attention). Chunk 0 runs cb-outer/dtile-inner
    accumulating all 6 projection outputs in 6 PSUM banks at once, so PE
    consumption tracks the DMA arrival order (geometric piece sizes,
    wt|xt interleaved across the two HWDGE rings) with minimal startup
    stall; later chunks run dtile-outer with the 6-buffer rotation.
    Q/K rows host-permuted per head into [even-pairs | odd-pairs] so
    rope works on partition halves. rope: ACT casts the psum to bf16
    (single PSUM read frees the bank), then 5 bf16 DVE TT ops at 2
    elem/cyc: t1 = pb*cos, t2 = pb*sin half-wise, dest = t1 -+ t2.
    V is PE-transposed back to [s, d]; V-path evacuations on ACT.
  phase B (per 512-col q chunk t, per head h):
    scores_T [k,q] = KT_tile.T @ QT, restricted to the causally live
    q range for diagonal k blocks; P_T = exp(scale*scores_T) on ACT
    (no max subtraction: |scores*scale| < ~10). The diagonal 128x128
    block gets a triangular 0/1 mask (DVE bf16). PV accumulates over
    restricted ranges; denominator partials accumulate in bf16 on DVE
    and are partition-reduced+broadcast by a ones matmul; normalize =
    reciprocal_approx_fast (custom DVE op, ~5x the stock reciprocal)
    + multiply -> at [d, s] bf16. t=0 (no oproj filler available) is
    emitted two-pass: scores+exp for heads 0-2 back-to-back, then each
    head's PV pass interleaves with the remaining scores so the
    recip/at chain latencies hide under score matmuls.
  o-proj: psum [s,512e] accumulated over the 4 heads, lhsT=at slices,
    rhs=woT [d', e]; evacuated (DVE/ACT alternating) to bf16 and DMA'd
    in pieces alternating the sync/scalar rings (512-col pieces for the
    final chunk to shrink the end-of-kernel drain). The o-projection
    for chunk t-1 is emitted between the per-head attention groups of
    chunk t (PE filler during exps).

DMA schedule: chunk-0 wt|xt pieces first (geometric sizes), then chunk-0
cos/sin, chunk-1 xt, remaining cos/sin, chunk-2/3 xt, woT last (phase B
needs it only at the first oproj group; its transfer lands in the quiet
late-phase-A window). The exp activation table is preloaded during the
initial DMA wait. Single-core time matches the TimelineSim cost model
(~358us, PE ~94% busy); the 8-core run adds ~70us from shared-HBM
contention and the sustained-load PE downclock.
"""

import numpy as np
import ml_dtypes

S = 2048
DIM = 4096
HD = 128
N_CORES = 8
QH_PER_CORE = 4  # 512 q dims per core
DQ = QH_PER_CORE * HD  # 512
SCALE = 1.0 / float(np.sqrt(HD))
SB = S // 128  # 16 s blocks
CB = DIM // 128  # 32 contraction blocks
NT = S // 512  # 4 q chunks
ET = DIM // 512  # 8 e tiles

bf16 = ml_dtypes.bfloat16

_RUNNER = None

ALL_STAGES = frozenset({"proj", "rope", "tpose", "attn", "oproj", "outdma"})


def _build(reps=None, stages=ALL_STAGES):
    import concourse.bass as bass
    import concourse.mybir as mybir
    import concourse.tile as tile
    from concourse import bacc
    from concourse.masks import make_identity
    from contextlib import nullcontext

    dt = mybir.dt
    Exp = mybir.ActivationFunctionType.Exp

    nc = bacc.Bacc(
        "TRN2", target_bir_lowering=False, debug=False, num_devices=N_CORES
    )

    xt_d = nc.dram_tensor("xt", [DIM, S], dt.bfloat16, kind="ExternalInput").ap()
    wt_d = nc.dram_tensor("wt", [DIM, 768], dt.bfloat16, kind="ExternalInput").ap()
    wot_d = nc.dram_tensor("wot", [DQ, DIM], dt.bfloat16, kind="ExternalInput").ap()
    csd_d = nc.dram_tensor("csd", [128, S], dt.bfloat16, kind="ExternalInput").ap()
    snd_d = nc.dram_tensor("snd", [128, S], dt.bfloat16, kind="ExternalInput").ap()
    tri_d = nc.dram_tensor("tri", [128, 128], dt.bfloat16, kind="ExternalInput").ap()
    out_d = nc.dram_tensor("out", [S, DIM], dt.bfloat16, kind="ExternalOutput").ap()

    xt_r = xt_d.rearrange("(cb c) s -> c cb s", c=128)
    wt_r = wt_d.rearrange("(cb c) n -> c cb n", c=128)

    with tile.TileContext(nc) as tc:
        with tc.For_i(0, reps, 1) if reps else nullcontext(), tc.tile_pool(
            name="const", bufs=1
        ) as cp:
            ones_sb = cp.tile([128, 128], dt.bfloat16)
            nc.vector.memset(ones_sb, 1.0)
            # preload the exp table set during the initial DMA wait
            warm_sb = cp.tile([128, 8], dt.float32)
            nc.scalar.activation(warm_sb, ones_sb[:, 0:8], Exp)
            ident_sb = cp.tile([128, 128], dt.bfloat16)
            make_identity(nc, ident_sb)
            tri_sb = cp.tile([128, 128], dt.bfloat16)

            qt_sb = cp.tile([128, QH_PER_CORE, S], dt.bfloat16)  # [d, h, s]
            kt_sb = cp.tile([128, S], dt.bfloat16)  # [d, s]
            v_sb = cp.tile([128, SB, HD], dt.bfloat16)  # [s128, sb, d]
            woT_sb = cp.tile([128, QH_PER_CORE, DIM], dt.bfloat16)
            wot_r = wot_d.rearrange("(db p) e -> p db e", p=128)

            # ---------------- phase A: projections + rope (direct QT) ---------
            with (
                tc.tile_pool(name="pa", bufs=2) as pa,
                tc.tile_pool(name="pap", bufs=3, space="PSUM") as pap,
            ):
                wt_sb = pa.tile([128, CB, 768], dt.bfloat16, bufs=1)
                csd_sb = pa.tile([128, S], dt.bfloat16, bufs=1)
                snd_sb = pa.tile([128, S], dt.bfloat16, bufs=1)
                xt0_sb = pa.tile([128, CB, 512], dt.bfloat16, name="xt0",
                                 tag="xt", bufs=2)
                # Startup DMA: per-cb interleaved (wt | xt) pieces so the
                # first matmul's operands (one cb block, ~320KB) land in ~1us
                # and chunk 0's cb-ordered consumption tracks arrival order.
                # cos/sin loads are split per chunk (only chunk 0's slice is
                # needed early); woT is queued behind the xt chunks (emitted
                # at sc==3) since phase B needs it much later.
                # geometric piece sizes: small first so the first matmuls'
                # operands land fast (the scheduler's wait coalescing makes
                # the first matmul wait out a few pieces regardless), larger
                # later so the ~630ns/piece HWDGE descriptor cost doesn't
                # throttle aggregate bandwidth.
                splits = [0, 1, 2, 4, 8, 12, 16, 20, 24, 28, 32]
                for i, (lo, hi) in enumerate(zip(splits[:-1], splits[1:])):
                    # first two xt pieces ride the sync ring right behind
                    # their wt pieces: the first matmuls' (coalesced) waits
                    # then cover only fast-arriving pieces
                    xt_eng = nc.sync if i < 2 else nc.scalar
                    nc.sync.dma_start(
                        out=wt_sb[:, lo:hi, :], in_=wt_r[:, lo:hi, :]
                    )
                    xt_eng.dma_start(
                        out=xt0_sb[:, lo:hi, :], in_=xt_r[:, lo:hi, 0:512]
                    )
                    if i == 2:
                        # tri + chunk-0 cos/sin behind the first piece-pairs
                        # (not needed before chunk-0's rope / t=0 scores)
                        nc.scalar.dma_start(out=tri_sb, in_=tri_d)
                        nc.scalar.dma_start(
                            out=csd_sb[:, 0:512], in_=csd_d[:, 0:512]
                        )
                        nc.scalar.dma_start(
                            out=snd_sb[:, 0:512], in_=snd_d[:, 0:512]
                        )

                def rope_evac(ps, dest, s0):
                    # dest[0:64]   = a*cos - b*sin
                    # dest[64:128] = a*sin + b*cos   (a=rows 0:64, b=rows 64:128)
                    # ACT evacuates the PSUM to bf16 (freeing the bank after a
                    # single read), then all products run as bf16 DVE TT ops in
                    # 2x_1P mode (2 elem/cyc) — half the fp32 cost. DVE 2-input
                    # SBUF ops need equal base partitions on the inputs (outputs
                    # may shift), so the sin products are computed half-wise
                    # with the cross-half move on the output side.
                    cs = csd_sb[:, s0 : s0 + 512]
                    sn = snd_sb[:, s0 : s0 + 512]
                    pb = pa.tile([128, 512], dt.bfloat16, tag="pb")
                    nc.scalar.copy(pb, ps)
                    t1 = pa.tile([128, 512], dt.bfloat16, tag="t1")
                    t2 = pa.tile([128, 512], dt.bfloat16, tag="t2")
                    nc.vector.tensor_mul(t1, pb, cs)
                    nc.vector.tensor_mul(t2[0:64, :], pb[64:128, :], sn[64:128, :])
                    nc.vector.tensor_mul(t2[64:128, :], pb[0:64, :], sn[0:64, :])
                    nc.vector.tensor_sub(dest[0:64, :], t1[0:64, :], t2[0:64, :])
                    nc.vector.tensor_add(dest[64:128, :], t1[64:128, :], t2[64:128, :])

                do_proj = stages & {"proj", "projcw", "projldw", "projnodma"}

                def wt_ap(cb, c0, c1):
                    return wt_sb[:, cb, c0:c1]

                def proj_evac(ps, dtile, sc, s0):
                    if dtile < QH_PER_CORE:
                        if "rope" in stages:
                            rope_evac(ps, qt_sb[:, dtile, s0 : s0 + 512], s0)
                    elif dtile == QH_PER_CORE:
                        if "rope" in stages:
                            rope_evac(ps, kt_sb[:, s0 : s0 + 512], s0)
                    elif "tpose" in stages:
                        vt_st = pa.tile([128, 512], dt.bfloat16, tag="vt")
                        nc.scalar.copy(vt_st, ps)
                        for b in range(4):
                            pst = pap.tile(
                                [128, 128], dt.bfloat16, tag="tp", bufs=2
                            )
                            nc.tensor.transpose(
                                pst, vt_st[:, b * 128 : (b + 1) * 128], ident_sb
                            )
                            nc.scalar.copy(v_sb[:, sc * 4 + b, :], pst)

                for sc in range(4):  # s chunks of 512
                    s0 = sc * 512
                    if sc == 0 or "projnodma" in stages:
                        xt_sb = xt0_sb
                    else:
                        xt_sb = pa.tile([128, CB, 512], dt.bfloat16,
                                        name=f"xt{sc}", tag="xt", bufs=2)
                        for half, eng in ((0, nc.sync), (1, nc.scalar)):
                            eng.dma_start(
                                out=xt_sb[:, 16 * half : 16 * (half + 1), :],
                                in_=xt_r[:, 16 * half : 16 * (half + 1), s0 : s0 + 512],
                            )
                        if sc == 1:
                            # remaining cos/sin slices, behind chunk 1's xt
                            nc.scalar.dma_start(
                                out=csd_sb[:, 512:S], in_=csd_d[:, 512:S]
                            )
                            nc.scalar.dma_start(
                                out=snd_sb[:, 512:S], in_=snd_d[:, 512:S]
                            )
                        if sc == 3:
                            # woT queued behind all xt chunks: transfers land
                            # in the quiet late-phase-A DMA window
                            for i, eng in enumerate(
                                (nc.sync, nc.scalar, nc.sync, nc.scalar)
                            ):
                                eng.dma_start(
                                    out=woT_sb[:, :, 1024 * i : 1024 * (i + 1)],
                                    in_=wot_r[:, :, 1024 * i : 1024 * (i + 1)],
                                )
                    if sc == 0 and do_proj:
                        # chunk 0 runs cb-outer/dtile-inner, accumulating all
                        # six projection outputs in six PSUM banks at once, so
                        # PE consumption tracks the per-cb DMA arrival order
                        # with no startup stall.
                        pss = [
                            pap.tile([128, 512], dt.float32, name=f"ps{d}",
                                     tag="proj", bufs=6)
                            for d in range(6)
                        ]
                        for cb in range(CB):
                            rhs = xt_sb[:, cb, :]
                            for dtile in range(6):
                                nc.tensor.matmul(
                                    pss[dtile],
                                    lhsT=wt_ap(cb, dtile * 128,
                                               (dtile + 1) * 128),
                                    rhs=rhs,
                                    start=(cb == 0),
                                    stop=(cb == CB - 1),
                                )
                        for dtile in range(6):
                            proj_evac(pss[dtile], dtile, sc, s0)
                        continue
                    for dtile in range(6):  # 4 Q heads, K, V
                        ps = pap.tile([128, 512], dt.float32, tag="proj", bufs=6)
                        const_w = "projcw" in stages
                        pre_ldw = "projldw" in stages
                        for ci, cb in enumerate(range(CB) if do_proj else []):
                            w_ap = wt_ap(0 if const_w else cb,
                                         dtile * 128, (dtile + 1) * 128)
                            if pre_ldw:
                                nc.tensor.ldweights(weights=w_ap)
                            nc.tensor.matmul(
                                ps,
                                lhsT=w_ap,
                                rhs=xt_sb[:, cb, :],
                                start=(ci == 0),
                                stop=(ci == CB - 1),
                            )
                        proj_evac(ps, dtile, sc, s0)

            # ---------------- phase B: attention + output projection ----------
            with (
                tc.tile_pool(name="pb", bufs=2) as pb,
                tc.tile_pool(name="pbp", bufs=2, space="PSUM") as pbp,
            ):
                def oproj_group(t, sbl, ats):
                    if "oproj" not in stages:
                        return
                    o_sb = pb.tile([128, DIM], dt.bfloat16, tag="osb", bufs=3)
                    row = (4 * t + sbl) * 128
                    # final-chunk groups flush per-e (512-col pieces) so the
                    # end-of-kernel DMA drain after the last evacuation is
                    # minimal; earlier groups use 1024-col pieces.
                    estep = 1 if t == NT - 1 else 2
                    for e in range(ET):
                        ps_out = pbp.tile([128, 512], dt.float32, tag="oproj")
                        for h in range(QH_PER_CORE):
                            nc.tensor.matmul(
                                ps_out,
                                lhsT=ats[h][:, sbl * 128 : (sbl + 1) * 128],
                                rhs=woT_sb[:, h, e * 512 : (e + 1) * 512],
                                start=(h == 0),
                                stop=(h == QH_PER_CORE - 1),
                            )
                        # split evacuations between DVE and ACT
                        ev_eng = nc.vector.tensor_copy if e % 2 == 0 else nc.scalar.copy
                        ev_eng(o_sb[:, e * 512 : (e + 1) * 512], ps_out)
                        # out DMA pieces alternate rings: keeps the
                        # end-of-group DMA tail short and halves per-ring
                        # burst pressure.
                        if (e + 1) % estep == 0 and "outdma" in stages:
                            lo = (e + 1 - estep) * 512
                            eng = nc.sync if (e // estep) % 2 == 0 else nc.scalar
                            eng.dma_start(
                                out=out_d[row : row + 128, lo : (e + 1) * 512],
                                in_=o_sb[:, lo : (e + 1) * 512],
                            )

                def scores_pair(t, h, qs, kb0, dacc):
                    """Scores matmuls + exp + diag mask + bf16 denominator
                    accumulation for one kb pair; returns the pt tile."""
                    diag = kb0 >= 4 * t
                    ps_s = pbp.tile([128, 1024], dt.float32, tag="scores")
                    for j in (0, 1):
                        kb = kb0 + j
                        qlo = 128 * (kb - 4 * t) if diag else 0
                        nc.tensor.matmul(
                            ps_s[:, j * 512 + qlo : (j + 1) * 512],
                            lhsT=kt_sb[:, kb * 128 : (kb + 1) * 128],
                            rhs=qs[:, qlo:512],
                            start=True,
                            stop=True,
                        )
                    pt = pb.tile([128, 1024], dt.bfloat16, tag="pt", bufs=8)
                    if diag:
                        # per-kb exp over exactly the written range
                        for j in (0, 1):
                            qlo = 128 * (kb0 + j - 4 * t)
                            nc.scalar.activation(
                                pt[:, j * 512 + qlo : (j + 1) * 512],
                                ps_s[:, j * 512 + qlo : (j + 1) * 512],
                                Exp,
                                scale=SCALE,
                            )
                    else:
                        nc.scalar.activation(pt, ps_s, Exp, scale=SCALE)
                    for j in (0, 1):
                        kb = kb0 + j
                        off = j * 512
                        if kb >= 4 * t:
                            qlo = 128 * (kb - 4 * t)
                            # mask the triangular 128x128 diagonal block
                            nc.vector.tensor_mul(
                                pt[:, off + qlo : off + qlo + 128],
                                pt[:, off + qlo : off + qlo + 128],
                                tri_sb,
                            )
                        else:
                            qlo = 0
                        # denominator partials accumulate in bf16 on DVE
                        if kb == 0:
                            nc.vector.tensor_copy(dacc, pt[:, 0:512])
                        else:
                            nc.vector.tensor_add(
                                dacc[:, qlo:512],
                                dacc[:, qlo:512],
                                pt[:, off + qlo : off + 512],
                            )
                    return pt

                def pv_pair(t, ps_o, nkb, kb0, pt):
                    for j in (0, 1):
                        kb = kb0 + j
                        off = j * 512
                        qlo = 128 * (kb - 4 * t) if kb >= 4 * t else 0
                        nc.tensor.matmul(
                            ps_o[:, qlo:512],
                            lhsT=v_sb[:, kb, :],
                            rhs=pt[:, off + qlo : off + 512],
                            start=(kb == 0),
                            stop=(kb == nkb - 1),
                        )

                def finish_head(t, h, ps_o, dacc, at_tiles):
                    # partition-reduce + broadcast the denominator in
                    # one matmul: every output row = the column sum
                    ps_d = pbp.tile([128, 512], dt.float32, tag="denom", bufs=1)
                    nc.tensor.matmul(
                        ps_d, lhsT=ones_sb, rhs=dacc, start=True, stop=True
                    )
                    recip = pb.tile([128, 512], dt.float32, tag="recip", bufs=3)
                    nc.vector.reciprocal_approx_fast(recip, ps_d)
                    at = pb.tile([128, 512], dt.bfloat16, name=f"at_{t}_{h}",
                                 tag=f"at{h}")
                    nc.vector.tensor_mul(at, ps_o, recip)
                    at_tiles.append(at)

                prev_ats = None
                for t in range(NT if "attn" in stages else 0):
                    nkb = 4 * (t + 1)
                    at_tiles = []
                    if t == 0:
                        # t=0 has no oproj filler between heads, so the
                        # recip/at chains would starve PE. Two-pass emission:
                        # scores+exp for heads 0-2 run back-to-back, then each
                        # head's PV pass interleaves with the remaining
                        # scores so chain latencies hide under score matmuls.
                        state = {}

                        def sc0(h):
                            qs = qt_sb[:, h, 0:512]
                            dacc = pb.tile([128, 512], dt.bfloat16,
                                           name=f"dacc0{h}", tag="dacc", bufs=4)
                            pts = [scores_pair(0, h, qs, kb0, dacc)
                                   for kb0 in (0, 2)]
                            state[h] = (dacc, pts)

                        def pv0(h):
                            dacc, pts = state[h]
                            ps_o = pbp.tile([128, 512], dt.float32,
                                            name=f"pso0{h}", tag="attnT",
                                            bufs=1)
                            for kb0, pt in zip((0, 2), pts):
                                pv_pair(0, ps_o, nkb, kb0, pt)
                            finish_head(0, h, ps_o, dacc, at_tiles)

                        sc0(0)
                        sc0(1)
                        sc0(2)
                        pv0(0)
                        sc0(3)
                        pv0(1)
                        pv0(2)
                        pv0(3)
                    else:
                        for h in range(QH_PER_CORE):
                            qs = qt_sb[:, h, t * 512 : (t + 1) * 512]
                            ps_o = pbp.tile([128, 512], dt.float32,
                                            tag="attnT", bufs=1)
                            dacc = pb.tile([128, 512], dt.bfloat16,
                                           tag="dacc", bufs=4)
                            for kb0 in range(0, nkb, 2):
                                pt = scores_pair(t, h, qs, kb0, dacc)
                                pv_pair(t, ps_o, nkb, kb0, pt)
                            finish_head(t, h, ps_o, dacc, at_tiles)
                            if prev_ats is not None:
                                oproj_group(t - 1, h, prev_ats)
                    prev_ats = at_tiles
                if prev_ats is not None:
                    for sbl in range(4):
                        oproj_group(NT - 1, sbl, prev_ats)
    nc.compile()
    return nc


def _prep_inputs(x, cos, sin, wq, wk, wv, wo):
    x = np.asarray(x, dtype=np.float32)
    cos = np.asarray(cos, dtype=np.float32)
    sin = np.asarray(sin, dtype=np.float32)
    wq = np.asarray(wq, dtype=np.float32)
    wk = np.asarray(wk, dtype=np.float32)
    wv = np.asarray(wv, dtype=np.float32)
    wo = np.asarray(wo, dtype=np.float32)

    xt = np.ascontiguousarray(x[0].T).astype(bf16)  # [DIM, S]
    # cos/sin transposed and duplicated into both partition halves [128, S]
    csd = np.ascontiguousarray(np.tile(cos.T, (2, 1)).astype(bf16))
    snd = np.ascontiguousarray(np.tile(sin.T, (2, 1)).astype(bf16))
    # de-interleave perm: head dim pairs (2i, 2i+1) -> rows (i, 64+i)
    perm = np.concatenate([np.arange(0, HD, 2), np.arange(1, HD, 2)])

    # causal triangular mask for a diagonal 128x128 block: keep k <= q
    r = np.arange(128)[:, None]
    c = np.arange(128)[None, :]
    tri = np.ascontiguousarray((r <= c).astype(bf16))

    in_maps = []
    for i in range(N_CORES):
        wq_i = wq[DQ * i : DQ * (i + 1)]  # [512, DIM]
        wk_i = wk[HD * i : HD * (i + 1)]  # [128, DIM]
        wv_i = wv[HD * i : HD * (i + 1)]
        wq_p = wq_i.reshape(QH_PER_CORE, HD, DIM)[:, perm, :].reshape(DQ, DIM)
        wk_p = wk_i[perm, :]
        wt = np.concatenate([wq_p.T, wk_p.T, wv_i.T], axis=1).astype(bf16)
        wot = np.ascontiguousarray(wo[:, DQ * i : DQ * (i + 1)].T).astype(
            bf16
        )  # [512, DIM]
        in_maps.append(
            {
                "xt": xt,
                "wt": np.ascontiguousarray(wt),
                "wot": wot,
                "csd": csd,
                "snd": snd,
                "tri": tri,
            }
        )
    return in_maps


def _get_runner():
    global _RUNNER
    if _RUNNER is None:
        _RUNNER = _build()
    return _RUNNER


def kernel(x, cos, sin, wq, wk, wv, wo):
    from concourse.bass_utils import run_bass_kernel_spmd

    nc = _get_runner()
    in_maps = _prep_inputs(x, cos, sin, wq, wk, wv, wo)
    res = run_bass_kernel_spmd(nc, in_maps, list(range(N_CORES)))
    out = np.zeros((S, DIM), dtype=np.float32)
    for i in range(N_CORES):
        out += np.asarray(res.results[i]["out"], dtype=np.float32)
    return out[None].astype(np.float32)



# revision 62
# speedup vs baseline: 1.0522x; 1.0522x over previous
"""Trainium2 Bass kernel for GQA attention prefill (Mistral-style, RoPE, causal).

B=1, S=2048, DIM=4096, 32 Q heads / 8 KV heads, HD=128, rope theta 1e6.

Sharding: tensor-parallel over heads across 8 cores. Core i gets Q heads
4i..4i+3 and KV head i. x is replicated (pre-transposed + bf16-cast on host).
Each core computes its 4 heads' attention and a partial output projection
(contraction over its 512 input dims of wo); the host sums the 8 partials
(partials are written bf16 to halve the output DMA).

Per-core dataflow (all matmuls bf16 with fp32 PSUM accumulation):
  phase A (per 512-col s chunk):
    xT tiles [c,s] (lhsT) x wT [c, q|k|v] (rhs) -> psum [d, s]
    (already transposed for attention). Chunk 0 runs cb-outer/dtile-inner
    accumulating all 6 projection outputs in 6 PSUM banks at once, so PE
    consumption tracks the DMA arrival order (geometric piece sizes,
    wt|xt interleaved across the two HWDGE rings) with minimal startup
    stall; later chunks run dtile-outer with the 6-buffer rotation.
    Q/K rows host-permuted per head into [even-pairs | odd-pairs] so
    rope works on partition halves. rope: ACT casts the psum to bf16
    (single PSUM read frees the bank), then 5 bf16 DVE TT ops at 2
    elem/cyc: t1 = pb*cos, t2 = pb*sin half-wise, dest = t1 -+ t2.
    V is PE-transposed back to [s, d]; V-path evacuations on ACT.
  phase B (per 512-col q chunk t, per head h):
    scores_T [k,q] = KT_tile.T @ QT, restricted to the causally live
    q range for diagonal k blocks; P_T = exp(scale*scores_T) on ACT
    (no max subtraction: |scores*scale| < ~10). The diagonal 128x128
    block gets a triangular 0/1 mask (DVE bf16). PV accumulates over
    restricted ranges; denominator partials accumulate in bf16 on DVE
    and are partition-reduced+broadcast by a ones matmul; normalize =
    reciprocal_approx_fast (custom DVE op, ~5x the stock reciprocal)
    + multiply -> at [d, s] bf16. t=0 (no oproj filler available) is
    emitted two-pass: scores+exp for heads 0-2 back-to-back, then each
    head's PV pass interleaves with the remaining scores so the
    recip/at chain latencies hide under score matmuls.
  o-proj: psum [s,512e] accumulated over the 4 heads, lhsT=at slices,
    rhs=woT [d', e]; evacuated (DVE/ACT alternating) to bf16 and DMA'd
    in pieces alternating the sync/scalar rings (512-col pieces for the
    final chunk to shrink the end-of-kernel drain). The o-projection
    for chunk t-1 is emitted between the per-head attention groups of
    chunk t (PE filler during exps).

DMA schedule: chunk-0 wt|xt pieces first (geometric sizes), then chunk-0
cos/sin, chunk-1 xt, remaining cos/sin, chunk-2/3 xt, woT last (phase B
needs it only at the first oproj group; its transfer lands in the quiet
late-phase-A window). The exp activation table is preloaded during the
initial DMA wait. Single-core time matches the TimelineSim cost model
(~358us, PE ~94% busy); the 8-core run adds ~70us from shared-HBM
contention and the sustained-load PE downclock.
"""

import numpy as np
import ml_dtypes

S = 2048
DIM = 4096
HD = 128
N_CORES = 8
QH_PER_CORE = 4  # 512 q dims per core
DQ = QH_PER_CORE * HD  # 512
SCALE = 1.0 / float(np.sqrt(HD))
SB = S // 128  # 16 s blocks
CB = DIM // 128  # 32 contraction blocks
NT = S // 512  # 4 q chunks
ET = DIM // 512  # 8 e tiles

bf16 = ml_dtypes.bfloat16

_RUNNER = None

ALL_STAGES = frozenset({"proj", "rope", "tpose", "attn", "oproj", "outdma"})


def _build(reps=None, stages=ALL_STAGES):
    import concourse.bass as bass
    import concourse.mybir as mybir
    import concourse.tile as tile
    from concourse import bacc
    from concourse.masks import make_identity
    from contextlib import nullcontext

    dt = mybir.dt
    Exp = mybir.ActivationFunctionType.Exp

    nc = bacc.Bacc(
        "TRN2", target_bir_lowering=False, debug=False, num_devices=N_CORES
    )

    xt_d = nc.dram_tensor("xt", [DIM, S], dt.bfloat16, kind="ExternalInput").ap()
    wt_d = nc.dram_tensor("wt", [DIM, 768], dt.bfloat16, kind="ExternalInput").ap()
    wot_d = nc.dram_tensor("wot", [DQ, DIM], dt.bfloat16, kind="ExternalInput").ap()
    csd_d = nc.dram_tensor("csd", [128, S], dt.bfloat16, kind="ExternalInput").ap()
    snd_d = nc.dram_tensor("snd", [128, S], dt.bfloat16, kind="ExternalInput").ap()
    tri_d = nc.dram_tensor("tri", [128, 128], dt.bfloat16, kind="ExternalInput").ap()
    out_d = nc.dram_tensor("out", [S, DIM], dt.bfloat16, kind="ExternalOutput").ap()

    xt_r = xt_d.rearrange("(cb c) s -> c cb s", c=128)
    wt_r = wt_d.rearrange("(cb c) n -> c cb n", c=128)

    with tile.TileContext(nc) as tc:
        with tc.For_i(0, reps, 1) if reps else nullcontext(), tc.tile_pool(
            name="const", bufs=1
        ) as cp:
            ones_sb = cp.tile([128, 128], dt.bfloat16)
            nc.vector.memset(ones_sb, 1.0)
            # preload the exp table set during the initial DMA wait
            warm_sb = cp.tile([128, 8], dt.float32)
            nc.scalar.activation(warm_sb, ones_sb[:, 0:8], Exp)
            ident_sb = cp.tile([128, 128], dt.bfloat16)
            make_identity(nc, ident_sb)
            tri_sb = cp.tile([128, 128], dt.bfloat16)

            qt_sb = cp.tile([128, QH_PER_CORE, S], dt.bfloat16)  # [d, h, s]
            kt_sb = cp.tile([128, S], dt.bfloat16)  # [d, s]
            v_sb = cp.tile([128, SB, HD], dt.bfloat16)  # [s128, sb, d]
            woT_sb = cp.tile([128, QH_PER_CORE, DIM], dt.bfloat16)
            wot_r = wot_d.rearrange("(db p) e -> p db e", p=128)

            # ---------------- phase A: projections + rope (direct QT) ---------
            with (
                tc.tile_pool(name="pa", bufs=2) as pa,
                tc.tile_pool(name="pap", bufs=3, space="PSUM") as pap,
            ):
                wt_sb = pa.tile([128, CB, 768], dt.bfloat16, bufs=1)
                csd_sb = pa.tile([128, S], dt.bfloat16, bufs=1)
                snd_sb = pa.tile([128, S], dt.bfloat16, bufs=1)
                xt0_sb = pa.tile([128, CB, 512], dt.bfloat16, name="xt0",
                                 tag="xt", bufs=2)
                # Startup DMA: per-cb interleaved (wt | xt) pieces so the
                # first matmul's operands (one cb block, ~320KB) land in ~1us
                # and chunk 0's cb-ordered consumption tracks arrival order.
                # cos/sin loads are split per chunk (only chunk 0's slice is
                # needed early); woT is queued behind the xt chunks (emitted
                # at sc==3) since phase B needs it much later.
                # geometric piece sizes: small first so the first matmuls'
                # operands land fast (the scheduler's wait coalescing makes
                # the first matmul wait out a few pieces regardless), larger
                # later so the ~630ns/piece HWDGE descriptor cost doesn't
                # throttle aggregate bandwidth.
                splits = [0, 1, 2, 4, 8, 12, 16, 20, 24, 28, 32]
                for i, (lo, hi) in enumerate(zip(splits[:-1], splits[1:])):
                    # first two xt pieces ride the sync ring right behind
                    # their wt pieces: the first matmuls' (coalesced) waits
                    # then cover only fast-arriving pieces
                    xt_eng = nc.sync if i < 2 else nc.scalar
                    nc.sync.dma_start(
                        out=wt_sb[:, lo:hi, :], in_=wt_r[:, lo:hi, :]
                    )
                    xt_eng.dma_start(
                        out=xt0_sb[:, lo:hi, :], in_=xt_r[:, lo:hi, 0:512]
                    )
                    if i == 2:
                        # tri + chunk-0 cos/sin behind the first piece-pairs
                        # (not needed before chunk-0's rope / t=0 scores)
                        nc.scalar.dma_start(out=tri_sb, in_=tri_d)
                        nc.scalar.dma_start(
                            out=csd_sb[:, 0:512], in_=csd_d[:, 0:512]
                        )
                        nc.scalar.dma_start(
                            out=snd_sb[:, 0:512], in_=snd_d[:, 0:512]
                        )

                def rope_evac(ps, dest, s0):
                    # dest[0:64]   = a*cos - b*sin
                    # dest[64:128] = a*sin + b*cos   (a=rows 0:64, b=rows 64:128)
                    # ACT evacuates the PSUM to bf16 (freeing the bank after a
                    # single read), then all products run as bf16 DVE TT ops in
                    # 2x_1P mode (2 elem/cyc) — half the fp32 cost. DVE 2-input
                    # SBUF ops need equal base partitions on the inputs (outputs
                    # may shift), so the sin products are computed half-wise
                    # with the cross-half move on the output side.
                    cs = csd_sb[:, s0 : s0 + 512]
                    sn = snd_sb[:, s0 : s0 + 512]
                    pb = pa.tile([128, 512], dt.bfloat16, tag="pb")
                    nc.scalar.copy(pb, ps)
                    t1 = pa.tile([128, 512], dt.bfloat16, tag="t1")
                    t2 = pa.tile([128, 512], dt.bfloat16, tag="t2")
                    nc.vector.tensor_mul(t1, pb, cs)
                    nc.vector.tensor_mul(t2[0:64, :], pb[64:128, :], sn[64:128, :])
                    nc.vector.tensor_mul(t2[64:128, :], pb[0:64, :], sn[0:64, :])
                    nc.vector.tensor_sub(dest[0:64, :], t1[0:64, :], t2[0:64, :])
                    nc.vector.tensor_add(dest[64:128, :], t1[64:128, :], t2[64:128, :])

                do_proj = stages & {"proj", "projcw", "projldw", "projnodma"}

                def wt_ap(cb, c0, c1):
                    return wt_sb[:, cb, c0:c1]

                def proj_evac(ps, dtile, sc, s0):
                    if dtile < QH_PER_CORE:
                        if "rope" in stages:
                            rope_evac(ps, qt_sb[:, dtile, s0 : s0 + 512], s0)
                    elif dtile == QH_PER_CORE:
                        if "rope" in stages:
                            rope_evac(ps, kt_sb[:, s0 : s0 + 512], s0)
                    elif "tpose" in stages:
                        vt_st = pa.tile([128, 512], dt.bfloat16, tag="vt")
                        nc.scalar.copy(vt_st, ps)
                        for b in range(4):
                            pst = pap.tile(
                                [128, 128], dt.bfloat16, tag="tp", bufs=2
                            )
                            nc.tensor.transpose(
                                pst, vt_st[:, b * 128 : (b + 1) * 128], ident_sb
                            )
                            nc.scalar.copy(v_sb[:, sc * 4 + b, :], pst)

                for sc in range(4):  # s chunks of 512
                    s0 = sc * 512
                    if sc == 0 or "projnodma" in stages:
                        xt_sb = xt0_sb
                    else:
                        xt_sb = pa.tile([128, CB, 512], dt.bfloat16,
                                        name=f"xt{sc}", tag="xt", bufs=2)
                        for half, eng in ((0, nc.sync), (1, nc.scalar)):
                            eng.dma_start(
                                out=xt_sb[:, 16 * half : 16 * (half + 1), :],
                                in_=xt_r[:, 16 * half : 16 * (half + 1), s0 : s0 + 512],
                            )
                        if sc == 1:
                            # remaining cos/sin slices, behind chunk 1's xt
                            nc.scalar.dma_start(
                                out=csd_sb[:, 512:S], in_=csd_d[:, 512:S]
                            )
                            nc.scalar.dma_start(
                                out=snd_sb[:, 512:S], in_=snd_d[:, 512:S]
                            )

                    if sc == 0 and do_proj:
                        # chunk 0 runs cb-outer/dtile-inner, accumulating all
                        # six projection outputs in six PSUM banks at once, so
                        # PE consumption tracks the per-cb DMA arrival order
                        # with no startup stall.
                        pss = [
                            pap.tile([128, 512], dt.float32, name=f"ps{d}",
                                     tag="proj", bufs=6)
                            for d in range(6)
                        ]
                        for cb in range(CB):
                            rhs = xt_sb[:, cb, :]
                            for dtile in range(6):
                                nc.tensor.matmul(
                                    pss[dtile],
                                    lhsT=wt_ap(cb, dtile * 128,
                                               (dtile + 1) * 128),
                                    rhs=rhs,
                                    start=(cb == 0),
                                    stop=(cb == CB - 1),
                                )
                        for dtile in range(6):
                            proj_evac(pss[dtile], dtile, sc, s0)
                        continue
                    for dtile in range(6):  # 4 Q heads, K, V
                        ps = pap.tile([128, 512], dt.float32, tag="proj", bufs=6)
                        const_w = "projcw" in stages
                        pre_ldw = "projldw" in stages
                        for ci, cb in enumerate(range(CB) if do_proj else []):
                            w_ap = wt_ap(0 if const_w else cb,
                                         dtile * 128, (dtile + 1) * 128)
                            if pre_ldw:
                                nc.tensor.ldweights(weights=w_ap)
                            nc.tensor.matmul(
                                ps,
                                lhsT=w_ap,
                                rhs=xt_sb[:, cb, :],
                                start=(ci == 0),
                                stop=(ci == CB - 1),
                            )
                        proj_evac(ps, dtile, sc, s0)

            # ---------------- phase B: attention + output projection ----------
            with (
                tc.tile_pool(name="pb", bufs=2) as pb,
                tc.tile_pool(name="pbp", bufs=2, space="PSUM") as pbp,
            ):
                # woT loads at phase-B start: the rings are empty here (t=0
                # emits no output DMA) and this moves 4MB (15% of the input
                # stream) out of the contended phase-A window. 8 e-ordered
                # pieces land ahead of the first oproj group's sweep.
                for i in range(ET):
                    eng = nc.sync if i % 2 == 0 else nc.scalar
                    eng.dma_start(
                        out=woT_sb[:, :, 512 * i : 512 * (i + 1)],
                        in_=wot_r[:, :, 512 * i : 512 * (i + 1)],
                    )
                def oproj_group(t, sbl, ats):
                    if "oproj" not in stages:
                        return
                    o_sb = pb.tile([128, DIM], dt.bfloat16, tag="osb", bufs=3)
                    row = (4 * t + sbl) * 128
                    # final-chunk groups flush per-e (512-col pieces) so the
                    # end-of-kernel DMA drain after the last evacuation is
                    # minimal; earlier groups use 1024-col pieces.
                    estep = 1 if t == NT - 1 else 2
                    for e in range(ET):
                        ps_out = pbp.tile([128, 512], dt.float32, tag="oproj")
                        for h in range(QH_PER_CORE):
                            nc.tensor.matmul(
                                ps_out,
                                lhsT=ats[h][:, sbl * 128 : (sbl + 1) * 128],
                                rhs=woT_sb[:, h, e * 512 : (e + 1) * 512],
                                start=(h == 0),
                                stop=(h == QH_PER_CORE - 1),
                            )
                        # split evacuations between DVE and ACT
                        ev_eng = nc.vector.tensor_copy if e % 2 == 0 else nc.scalar.copy
                        ev_eng(o_sb[:, e * 512 : (e + 1) * 512], ps_out)
                        # out DMA pieces alternate rings: keeps the
                        # end-of-group DMA tail short and halves per-ring
                        # burst pressure.
                        if (e + 1) % estep == 0 and "outdma" in stages:
                            lo = (e + 1 - estep) * 512
                            eng = nc.sync if (e // estep) % 2 == 0 else nc.scalar
                            eng.dma_start(
                                out=out_d[row : row + 128, lo : (e + 1) * 512],
                                in_=o_sb[:, lo : (e + 1) * 512],
                            )

                def scores_pair(t, h, qs, kb0, dacc):
                    """Scores matmuls + exp + diag mask + bf16 denominator
                    accumulation for one kb pair; returns the pt tile."""
                    diag = kb0 >= 4 * t
                    ps_s = pbp.tile([128, 1024], dt.float32, tag="scores")
                    for j in (0, 1):
                        kb = kb0 + j
                        qlo = 128 * (kb - 4 * t) if diag else 0
                        nc.tensor.matmul(
                            ps_s[:, j * 512 + qlo : (j + 1) * 512],
                            lhsT=kt_sb[:, kb * 128 : (kb + 1) * 128],
                            rhs=qs[:, qlo:512],
                            start=True,
                            stop=True,
                        )
                    pt = pb.tile([128, 1024], dt.bfloat16, tag="pt", bufs=8)
                    if diag:
                        # per-kb exp over exactly the written range
                        for j in (0, 1):
                            qlo = 128 * (kb0 + j - 4 * t)
                            nc.scalar.activation(
                                pt[:, j * 512 + qlo : (j + 1) * 512],
                                ps_s[:, j * 512 + qlo : (j + 1) * 512],
                                Exp,
                                scale=SCALE,
                            )
                    else:
                        nc.scalar.activation(pt, ps_s, Exp, scale=SCALE)
                    for j in (0, 1):
                        kb = kb0 + j
                        off = j * 512
                        if kb >= 4 * t:
                            qlo = 128 * (kb - 4 * t)
                            # mask the triangular 128x128 diagonal block
                            nc.vector.tensor_mul(
                                pt[:, off + qlo : off + qlo + 128],
                                pt[:, off + qlo : off + qlo + 128],
                                tri_sb,
                            )
                        else:
                            qlo = 0
                        # denominator partials accumulate in bf16 on DVE
                        if kb == 0:
                            nc.vector.tensor_copy(dacc, pt[:, 0:512])
                        else:
                            nc.vector.tensor_add(
                                dacc[:, qlo:512],
                                dacc[:, qlo:512],
                                pt[:, off + qlo : off + 512],
                            )
                    return pt

                def pv_pair(t, ps_o, nkb, kb0, pt):
                    for j in (0, 1):
                        kb = kb0 + j
                        off = j * 512
                        qlo = 128 * (kb - 4 * t) if kb >= 4 * t else 0
                        nc.tensor.matmul(
                            ps_o[:, qlo:512],
                            lhsT=v_sb[:, kb, :],
                            rhs=pt[:, off + qlo : off + 512],
                            start=(kb == 0),
                            stop=(kb == nkb - 1),
                        )

                def finish_head(t, h, ps_o, dacc, at_tiles):
                    # partition-reduce + broadcast the denominator in
                    # one matmul: every output row = the column sum
                    ps_d = pbp.tile([128, 512], dt.float32, tag="denom", bufs=1)
                    nc.tensor.matmul(
                        ps_d, lhsT=ones_sb, rhs=dacc, start=True, stop=True
                    )
                    recip = pb.tile([128, 512], dt.float32, tag="recip", bufs=3)
                    nc.vector.reciprocal_approx_fast(recip, ps_d)
                    at = pb.tile([128, 512], dt.bfloat16, name=f"at_{t}_{h}",
                                 tag=f"at{h}")
                    nc.vector.tensor_mul(at, ps_o, recip)
                    at_tiles.append(at)

                prev_ats = None
                for t in range(NT if "attn" in stages else 0):
                    nkb = 4 * (t + 1)
                    at_tiles = []
                    if t == 0:
                        # t=0 has no oproj filler between heads, so the
                        # recip/at chains would starve PE. Two-pass emission:
                        # scores+exp for heads 0-2 run back-to-back, then each
                        # head's PV pass interleaves with the remaining
                        # scores so chain latencies hide under score matmuls.
                        state = {}

                        def sc0(h):
                            qs = qt_sb[:, h, 0:512]
                            dacc = pb.tile([128, 512], dt.bfloat16,
                                           name=f"dacc0{h}", tag="dacc", bufs=4)
                            pts = [scores_pair(0, h, qs, kb0, dacc)
                                   for kb0 in (0, 2)]
                            state[h] = (dacc, pts)

                        def pv0(h):
                            dacc, pts = state[h]
                            ps_o = pbp.tile([128, 512], dt.float32,
                                            name=f"pso0{h}", tag="attnT",
                                            bufs=1)
                            for kb0, pt in zip((0, 2), pts):
                                pv_pair(0, ps_o, nkb, kb0, pt)
                            finish_head(0, h, ps_o, dacc, at_tiles)

                        sc0(0)
                        sc0(1)
                        sc0(2)
                        pv0(0)
                        sc0(3)
                        pv0(1)
                        pv0(2)
                        pv0(3)
                    else:
                        for h in range(QH_PER_CORE):
                            qs = qt_sb[:, h, t * 512 : (t + 1) * 512]
                            ps_o = pbp.tile([128, 512], dt.float32,
                                            tag="attnT", bufs=1)
                            dacc = pb.tile([128, 512], dt.bfloat16,
                                           tag="dacc", bufs=4)
                            for kb0 in range(0, nkb, 2):
                                pt = scores_pair(t, h, qs, kb0, dacc)
                                pv_pair(t, ps_o, nkb, kb0, pt)
                            finish_head(t, h, ps_o, dacc, at_tiles)
                            if prev_ats is not None:
                                oproj_group(t - 1, h, prev_ats)
                    prev_ats = at_tiles
                if prev_ats is not None:
                    for sbl in range(4):
                        oproj_group(NT - 1, sbl, prev_ats)
    nc.compile()
    return nc


def _prep_inputs(x, cos, sin, wq, wk, wv, wo):
    x = np.asarray(x, dtype=np.float32)
    cos = np.asarray(cos, dtype=np.float32)
    sin = np.asarray(sin, dtype=np.float32)
    wq = np.asarray(wq, dtype=np.float32)
    wk = np.asarray(wk, dtype=np.float32)
    wv = np.asarray(wv, dtype=np.float32)
    wo = np.asarray(wo, dtype=np.float32)

    xt = np.ascontiguousarray(x[0].T).astype(bf16)  # [DIM, S]
    # cos/sin transposed and duplicated into both partition halves [128, S]
    csd = np.ascontiguousarray(np.tile(cos.T, (2, 1)).astype(bf16))
    snd = np.ascontiguousarray(np.tile(sin.T, (2, 1)).astype(bf16))
    # de-interleave perm: head dim pairs (2i, 2i+1) -> rows (i, 64+i)
    perm = np.concatenate([np.arange(0, HD, 2), np.arange(1, HD, 2)])

    # causal triangular mask for a diagonal 128x128 block: keep k <= q
    r = np.arange(128)[:, None]
    c = np.arange(128)[None, :]
    tri = np.ascontiguousarray((r <= c).astype(bf16))

    in_maps = []
    for i in range(N_CORES):
        wq_i = wq[DQ * i : DQ * (i + 1)]  # [512, DIM]
        wk_i = wk[HD * i : HD * (i + 1)]  # [128, DIM]
        wv_i = wv[HD * i : HD * (i + 1)]
        wq_p = wq_i.reshape(QH_PER_CORE, HD, DIM)[:, perm, :].reshape(DQ, DIM)
        wk_p = wk_i[perm, :]
        wt = np.concatenate([wq_p.T, wk_p.T, wv_i.T], axis=1).astype(bf16)
        wot = np.ascontiguousarray(wo[:, DQ * i : DQ * (i + 1)].T).astype(
            bf16
        )  # [512, DIM]
        in_maps.append(
            {
                "xt": xt,
                "wt": np.ascontiguousarray(wt),
                "wot": wot,
                "csd": csd,
                "snd": snd,
                "tri": tri,
            }
        )
    return in_maps


def _get_runner():
    global _RUNNER
    if _RUNNER is None:
        _RUNNER = _build()
    return _RUNNER


def kernel(x, cos, sin, wq, wk, wv, wo):
    from concourse.bass_utils import run_bass_kernel_spmd

    nc = _get_runner()
    in_maps = _prep_inputs(x, cos, sin, wq, wk, wv, wo)
    res = run_bass_kernel_spmd(nc, in_maps, list(range(N_CORES)))
    out = np.zeros((S, DIM), dtype=np.float32)
    for i in range(N_CORES):
        out += np.asarray(res.results[i]["out"], dtype=np.float32)
    return out[None].astype(np.float32)



# revision 63
# speedup vs baseline: 1.0559x; 1.0035x over previous
"""Trainium2 Bass kernel for GQA attention prefill (Mistral-style, RoPE, causal).

B=1, S=2048, DIM=4096, 32 Q heads / 8 KV heads, HD=128, rope theta 1e6.

Sharding: tensor-parallel over heads across 8 cores. Core i gets Q heads
4i..4i+3 and KV head i. x is replicated (pre-transposed + bf16-cast on host).
Each core computes its 4 heads' attention and a partial output projection
(contraction over its 512 input dims of wo); the host sums the 8 partials
(partials are written bf16 to halve the output DMA).

Per-core dataflow (all matmuls bf16 with fp32 PSUM accumulation):
  phase A (per 512-col s chunk):
    xT tiles [c,s] (lhsT) x wT [c, q|k|v] (rhs) -> psum [d, s]
    (already transposed for attention). Chunk 0 runs cb-outer/dtile-inner
    accumulating all 6 projection outputs in 6 PSUM banks at once, so PE
    consumption tracks the DMA arrival order (geometric piece sizes,
    wt|xt interleaved across the two HWDGE rings) with minimal startup
    stall; later chunks run dtile-outer with the 6-buffer rotation.
    Q/K rows host-permuted per head into [even-pairs | odd-pairs] so
    rope works on partition halves. rope: ACT casts the psum to bf16
    (single PSUM read frees the bank), then 5 bf16 DVE TT ops at 2
    elem/cyc: t1 = pb*cos, t2 = pb*sin half-wise, dest = t1 -+ t2.
    V is PE-transposed back to [s, d]; V-path evacuations on ACT.
  phase B (per 512-col q chunk t, per head h):
    scores_T [k,q] = KT_tile.T @ QT, restricted to the causally live
    q range for diagonal k blocks; P_T = exp(scale*scores_T) on ACT
    (no max subtraction: |scores*scale| < ~10). The diagonal 128x128
    block gets a triangular 0/1 mask (DVE bf16). PV accumulates over
    restricted ranges; denominator partials accumulate in bf16 on DVE
    and are partition-reduced+broadcast by a ones matmul; normalize =
    reciprocal_approx_fast (custom DVE op, ~5x the stock reciprocal)
    + multiply -> at [d, s] bf16. t=0 (no oproj filler available) is
    emitted two-pass: scores+exp for heads 0-2 back-to-back, then each
    head's PV pass interleaves with the remaining scores so the
    recip/at chain latencies hide under score matmuls.
  o-proj: psum [s,512e] accumulated over the 4 heads, lhsT=at slices,
    rhs=woT [d', e]; evacuated (DVE/ACT alternating) to bf16 and DMA'd
    in pieces alternating the sync/scalar rings (512-col pieces for the
    final chunk to shrink the end-of-kernel drain). The o-projection
    for chunk t-1 is emitted between the per-head attention groups of
    chunk t (PE filler during exps).

DMA schedule: chunk-0 wt|xt pieces first (geometric sizes), then chunk-0
cos/sin, chunk-1 xt, remaining cos/sin, chunk-2/3 xt, woT last (phase B
needs it only at the first oproj group; its transfer lands in the quiet
late-phase-A window). The exp activation table is preloaded during the
initial DMA wait. Single-core time matches the TimelineSim cost model
(~358us, PE ~94% busy); the 8-core run adds ~70us from shared-HBM
contention and the sustained-load PE downclock.
"""

import numpy as np
import ml_dtypes

S = 2048
DIM = 4096
HD = 128
N_CORES = 8
QH_PER_CORE = 4  # 512 q dims per core
DQ = QH_PER_CORE * HD  # 512
SCALE = 1.0 / float(np.sqrt(HD))
SB = S // 128  # 16 s blocks
CB = DIM // 128  # 32 contraction blocks
NT = S // 512  # 4 q chunks
ET = DIM // 512  # 8 e tiles

bf16 = ml_dtypes.bfloat16

_RUNNER = None

ALL_STAGES = frozenset({"proj", "rope", "tpose", "attn", "oproj", "outdma"})


def _build(reps=None, stages=ALL_STAGES):
    import concourse.bass as bass
    import concourse.mybir as mybir
    import concourse.tile as tile
    from concourse import bacc
    from concourse.masks import make_identity
    from contextlib import nullcontext

    dt = mybir.dt
    Exp = mybir.ActivationFunctionType.Exp

    nc = bacc.Bacc(
        "TRN2", target_bir_lowering=False, debug=False, num_devices=N_CORES
    )

    xt_d = nc.dram_tensor("xt", [DIM, S], dt.bfloat16, kind="ExternalInput").ap()
    wt_d = nc.dram_tensor("wt", [DIM, 768], dt.bfloat16, kind="ExternalInput").ap()
    wot_d = nc.dram_tensor("wot", [DQ, DIM], dt.bfloat16, kind="ExternalInput").ap()
    csd_d = nc.dram_tensor("csd", [128, S], dt.bfloat16, kind="ExternalInput").ap()
    snd_d = nc.dram_tensor("snd", [128, S], dt.bfloat16, kind="ExternalInput").ap()
    tri_d = nc.dram_tensor("tri", [128, 128], dt.bfloat16, kind="ExternalInput").ap()
    out_d = nc.dram_tensor("out", [S, DIM], dt.bfloat16, kind="ExternalOutput").ap()

    xt_r = xt_d.rearrange("(cb c) s -> c cb s", c=128)
    wt_r = wt_d.rearrange("(cb c) n -> c cb n", c=128)

    with tile.TileContext(nc) as tc:
        with tc.For_i(0, reps, 1) if reps else nullcontext(), tc.tile_pool(
            name="const", bufs=1
        ) as cp:
            ones_sb = cp.tile([128, 128], dt.bfloat16)
            nc.vector.memset(ones_sb, 1.0)
            # preload the exp table set during the initial DMA wait
            warm_sb = cp.tile([128, 8], dt.float32)
            nc.scalar.activation(warm_sb, ones_sb[:, 0:8], Exp)
            ident_sb = cp.tile([128, 128], dt.bfloat16)
            make_identity(nc, ident_sb)
            tri_sb = cp.tile([128, 128], dt.bfloat16)

            qt_sb = cp.tile([128, QH_PER_CORE, S], dt.bfloat16)  # [d, h, s]
            kt_sb = cp.tile([128, S], dt.bfloat16)  # [d, s]
            v_sb = cp.tile([128, SB, HD], dt.bfloat16)  # [s128, sb, d]
            woT_sb = cp.tile([128, QH_PER_CORE, DIM], dt.bfloat16)
            wot_r = wot_d.rearrange("(db p) e -> p db e", p=128)

            # ---------------- phase A: projections + rope (direct QT) ---------
            with (
                tc.tile_pool(name="pa", bufs=2) as pa,
                tc.tile_pool(name="pap", bufs=3, space="PSUM") as pap,
            ):
                wt_sb = pa.tile([128, CB, 768], dt.bfloat16, bufs=1)
                csd_sb = pa.tile([128, S], dt.bfloat16, bufs=1)
                snd_sb = pa.tile([128, S], dt.bfloat16, bufs=1)
                xt0_sb = pa.tile([128, CB, 512], dt.bfloat16, name="xt0",
                                 tag="xt", bufs=2)
                # Startup DMA: per-cb interleaved (wt | xt) pieces so the
                # first matmul's operands (one cb block, ~320KB) land in ~1us
                # and chunk 0's cb-ordered consumption tracks arrival order.
                # cos/sin loads are split per chunk (only chunk 0's slice is
                # needed early); woT is queued behind the xt chunks (emitted
                # at sc==3) since phase B needs it much later.
                # geometric piece sizes: small first so the first matmuls'
                # operands land fast (the scheduler's wait coalescing makes
                # the first matmul wait out a few pieces regardless), larger
                # later so the ~630ns/piece HWDGE descriptor cost doesn't
                # throttle aggregate bandwidth.
                splits = [0, 1, 2, 4, 8, 12, 16, 20, 24, 28, 32]
                for i, (lo, hi) in enumerate(zip(splits[:-1], splits[1:])):
                    # first two xt pieces ride the sync ring right behind
                    # their wt pieces: the first matmuls' (coalesced) waits
                    # then cover only fast-arriving pieces
                    xt_eng = nc.sync if i < 2 else nc.scalar
                    nc.sync.dma_start(
                        out=wt_sb[:, lo:hi, :], in_=wt_r[:, lo:hi, :]
                    )
                    xt_eng.dma_start(
                        out=xt0_sb[:, lo:hi, :], in_=xt_r[:, lo:hi, 0:512]
                    )
                    if i == 2:
                        # tri + chunk-0 cos/sin behind the first piece-pairs
                        # (not needed before chunk-0's rope / t=0 scores)
                        nc.scalar.dma_start(out=tri_sb, in_=tri_d)
                        nc.scalar.dma_start(
                            out=csd_sb[:, 0:512], in_=csd_d[:, 0:512]
                        )
                        nc.scalar.dma_start(
                            out=snd_sb[:, 0:512], in_=snd_d[:, 0:512]
                        )

                def rope_evac(ps, dest, s0):
                    # dest[0:64]   = a*cos - b*sin
                    # dest[64:128] = a*sin + b*cos   (a=rows 0:64, b=rows 64:128)
                    # ACT evacuates the PSUM to bf16 (freeing the bank after a
                    # single read), then all products run as bf16 DVE TT ops in
                    # 2x_1P mode (2 elem/cyc) — half the fp32 cost. DVE 2-input
                    # SBUF ops need equal base partitions on the inputs (outputs
                    # may shift), so the sin products are computed half-wise
                    # with the cross-half move on the output side.
                    cs = csd_sb[:, s0 : s0 + 512]
                    sn = snd_sb[:, s0 : s0 + 512]
                    pb = pa.tile([128, 512], dt.bfloat16, tag="pb")
                    nc.scalar.copy(pb, ps)
                    t1 = pa.tile([128, 512], dt.bfloat16, tag="t1")
                    t2 = pa.tile([128, 512], dt.bfloat16, tag="t2")
                    nc.vector.tensor_mul(t1, pb, cs)
                    nc.vector.tensor_mul(t2[0:64, :], pb[64:128, :], sn[64:128, :])
                    nc.vector.tensor_mul(t2[64:128, :], pb[0:64, :], sn[0:64, :])
                    nc.vector.tensor_sub(dest[0:64, :], t1[0:64, :], t2[0:64, :])
                    nc.vector.tensor_add(dest[64:128, :], t1[64:128, :], t2[64:128, :])

                do_proj = stages & {"proj", "projcw", "projldw", "projnodma"}

                def wt_ap(cb, c0, c1):
                    return wt_sb[:, cb, c0:c1]

                def proj_evac(ps, dtile, sc, s0):
                    if dtile < QH_PER_CORE:
                        if "rope" in stages:
                            rope_evac(ps, qt_sb[:, dtile, s0 : s0 + 512], s0)
                    elif dtile == QH_PER_CORE:
                        if "rope" in stages:
                            rope_evac(ps, kt_sb[:, s0 : s0 + 512], s0)
                    elif "tpose" in stages:
                        vt_st = pa.tile([128, 512], dt.bfloat16, tag="vt")
                        nc.scalar.copy(vt_st, ps)
                        for b in range(4):
                            pst = pap.tile(
                                [128, 128], dt.bfloat16, tag="tp", bufs=2
                            )
                            nc.tensor.transpose(
                                pst, vt_st[:, b * 128 : (b + 1) * 128], ident_sb
                            )
                            nc.scalar.copy(v_sb[:, sc * 4 + b, :], pst)

                for sc in range(4):  # s chunks of 512
                    s0 = sc * 512
                    if sc == 0 or "projnodma" in stages:
                        xt_sb = xt0_sb
                    else:
                        xt_sb = pa.tile([128, CB, 512], dt.bfloat16,
                                        name=f"xt{sc}", tag="xt", bufs=2)
                        for half, eng in ((0, nc.sync), (1, nc.scalar)):
                            eng.dma_start(
                                out=xt_sb[:, 16 * half : 16 * (half + 1), :],
                                in_=xt_r[:, 16 * half : 16 * (half + 1), s0 : s0 + 512],
                            )
                        if sc == 1:
                            # remaining cos/sin slices, behind chunk 1's xt
                            nc.scalar.dma_start(
                                out=csd_sb[:, 512:S], in_=csd_d[:, 512:S]
                            )
                            nc.scalar.dma_start(
                                out=snd_sb[:, 512:S], in_=snd_d[:, 512:S]
                            )

                    if sc == 0 and do_proj:
                        # chunk 0 runs cb-outer/dtile-inner, accumulating all
                        # six projection outputs in six PSUM banks at once, so
                        # PE consumption tracks the per-cb DMA arrival order
                        # with no startup stall.
                        pss = [
                            pap.tile([128, 512], dt.float32, name=f"ps{d}",
                                     tag="proj", bufs=6)
                            for d in range(6)
                        ]
                        for cb in range(CB):
                            rhs = xt_sb[:, cb, :]
                            for dtile in range(6):
                                nc.tensor.matmul(
                                    pss[dtile],
                                    lhsT=wt_ap(cb, dtile * 128,
                                               (dtile + 1) * 128),
                                    rhs=rhs,
                                    start=(cb == 0),
                                    stop=(cb == CB - 1),
                                )
                        for dtile in range(6):
                            proj_evac(pss[dtile], dtile, sc, s0)
                        continue
                    for dtile in range(6):  # 4 Q heads, K, V
                        ps = pap.tile([128, 512], dt.float32, tag="proj", bufs=6)
                        const_w = "projcw" in stages
                        pre_ldw = "projldw" in stages
                        for ci, cb in enumerate(range(CB) if do_proj else []):
                            w_ap = wt_ap(0 if const_w else cb,
                                         dtile * 128, (dtile + 1) * 128)
                            if pre_ldw:
                                nc.tensor.ldweights(weights=w_ap)
                            nc.tensor.matmul(
                                ps,
                                lhsT=w_ap,
                                rhs=xt_sb[:, cb, :],
                                start=(ci == 0),
                                stop=(ci == CB - 1),
                            )
                        proj_evac(ps, dtile, sc, s0)

            # ---------------- phase B: attention + output projection ----------
            with (
                tc.tile_pool(name="pb", bufs=2) as pb,
                tc.tile_pool(name="pbp", bufs=2, space="PSUM") as pbp,
            ):
                # woT loads at phase-B start: the rings are empty here (t=0
                # emits no output DMA) and this moves 4MB (15% of the input
                # stream) out of the contended phase-A window. 8 e-ordered
                # pieces land ahead of the first oproj group's sweep.
                for i in range(ET):
                    eng = nc.sync if i % 2 == 0 else nc.scalar
                    eng.dma_start(
                        out=woT_sb[:, :, 512 * i : 512 * (i + 1)],
                        in_=wot_r[:, :, 512 * i : 512 * (i + 1)],
                    )
                def oproj_group(t, sbl, ats):
                    if "oproj" not in stages:
                        return
                    # 5 staging buffers: decouples evacuations (and through
                    # the psum rotation, PE) from contended out-DMA latency
                    # at 8 cores — with 3, a group's evacs waited on the DMA
                    # of the group three back.
                    o_sb = pb.tile([128, DIM], dt.bfloat16, tag="osb", bufs=5)
                    row = (4 * t + sbl) * 128
                    # final-chunk groups flush per-e (512-col pieces) so the
                    # end-of-kernel DMA drain after the last evacuation is
                    # minimal; earlier groups use 1024-col pieces.
                    estep = 1 if t == NT - 1 else 2
                    for e in range(ET):
                        ps_out = pbp.tile([128, 512], dt.float32, tag="oproj")
                        for h in range(QH_PER_CORE):
                            nc.tensor.matmul(
                                ps_out,
                                lhsT=ats[h][:, sbl * 128 : (sbl + 1) * 128],
                                rhs=woT_sb[:, h, e * 512 : (e + 1) * 512],
                                start=(h == 0),
                                stop=(h == QH_PER_CORE - 1),
                            )
                        # split evacuations between DVE and ACT
                        ev_eng = nc.vector.tensor_copy if e % 2 == 0 else nc.scalar.copy
                        ev_eng(o_sb[:, e * 512 : (e + 1) * 512], ps_out)
                        # out DMA pieces alternate rings: keeps the
                        # end-of-group DMA tail short and halves per-ring
                        # burst pressure.
                        if (e + 1) % estep == 0 and "outdma" in stages:
                            lo = (e + 1 - estep) * 512
                            eng = nc.sync if (e // estep) % 2 == 0 else nc.scalar
                            eng.dma_start(
                                out=out_d[row : row + 128, lo : (e + 1) * 512],
                                in_=o_sb[:, lo : (e + 1) * 512],
                            )

                def scores_pair(t, h, qs, kb0, dacc):
                    """Scores matmuls + exp + diag mask + bf16 denominator
                    accumulation for one kb pair; returns the pt tile."""
                    diag = kb0 >= 4 * t
                    ps_s = pbp.tile([128, 1024], dt.float32, tag="scores")
                    for j in (0, 1):
                        kb = kb0 + j
                        qlo = 128 * (kb - 4 * t) if diag else 0
                        nc.tensor.matmul(
                            ps_s[:, j * 512 + qlo : (j + 1) * 512],
                            lhsT=kt_sb[:, kb * 128 : (kb + 1) * 128],
                            rhs=qs[:, qlo:512],
                            start=True,
                            stop=True,
                        )
                    pt = pb.tile([128, 1024], dt.bfloat16, tag="pt", bufs=8)
                    if diag:
                        # per-kb exp over exactly the written range
                        for j in (0, 1):
                            qlo = 128 * (kb0 + j - 4 * t)
                            nc.scalar.activation(
                                pt[:, j * 512 + qlo : (j + 1) * 512],
                                ps_s[:, j * 512 + qlo : (j + 1) * 512],
                                Exp,
                                scale=SCALE,
                            )
                    else:
                        nc.scalar.activation(pt, ps_s, Exp, scale=SCALE)
                    for j in (0, 1):
                        kb = kb0 + j
                        off = j * 512
                        if kb >= 4 * t:
                            qlo = 128 * (kb - 4 * t)
                            # mask the triangular 128x128 diagonal block
                            nc.vector.tensor_mul(
                                pt[:, off + qlo : off + qlo + 128],
                                pt[:, off + qlo : off + qlo + 128],
                                tri_sb,
                            )
                        else:
                            qlo = 0
                        # denominator partials accumulate in bf16 on DVE
                        if kb == 0:
                            nc.vector.tensor_copy(dacc, pt[:, 0:512])
                        else:
                            nc.vector.tensor_add(
                                dacc[:, qlo:512],
                                dacc[:, qlo:512],
                                pt[:, off + qlo : off + 512],
                            )
                    return pt

                def pv_pair(t, ps_o, nkb, kb0, pt):
                    for j in (0, 1):
                        kb = kb0 + j
                        off = j * 512
                        qlo = 128 * (kb - 4 * t) if kb >= 4 * t else 0
                        nc.tensor.matmul(
                            ps_o[:, qlo:512],
                            lhsT=v_sb[:, kb, :],
                            rhs=pt[:, off + qlo : off + 512],
                            start=(kb == 0),
                            stop=(kb == nkb - 1),
                        )

                def finish_head(t, h, ps_o, dacc, at_tiles):
                    # partition-reduce + broadcast the denominator in
                    # one matmul: every output row = the column sum
                    ps_d = pbp.tile([128, 512], dt.float32, tag="denom", bufs=1)
                    nc.tensor.matmul(
                        ps_d, lhsT=ones_sb, rhs=dacc, start=True, stop=True
                    )
                    recip = pb.tile([128, 512], dt.float32, tag="recip", bufs=3)
                    nc.vector.reciprocal_approx_fast(recip, ps_d)
                    at = pb.tile([128, 512], dt.bfloat16, name=f"at_{t}_{h}",
                                 tag=f"at{h}")
                    nc.vector.tensor_mul(at, ps_o, recip)
                    at_tiles.append(at)

                prev_ats = None
                for t in range(NT if "attn" in stages else 0):
                    nkb = 4 * (t + 1)
                    at_tiles = []
                    if t == 0:
                        # t=0 has no oproj filler between heads, so the
                        # recip/at chains would starve PE. Two-pass emission:
                        # scores+exp for heads 0-2 run back-to-back, then each
                        # head's PV pass interleaves with the remaining
                        # scores so chain latencies hide under score matmuls.
                        state = {}

                        def sc0(h):
                            qs = qt_sb[:, h, 0:512]
                            dacc = pb.tile([128, 512], dt.bfloat16,
                                           name=f"dacc0{h}", tag="dacc", bufs=4)
                            pts = [scores_pair(0, h, qs, kb0, dacc)
                                   for kb0 in (0, 2)]
                            state[h] = (dacc, pts)

                        def pv0(h):
                            dacc, pts = state[h]
                            ps_o = pbp.tile([128, 512], dt.float32,
                                            name=f"pso0{h}", tag="attnT",
                                            bufs=1)
                            for kb0, pt in zip((0, 2), pts):
                                pv_pair(0, ps_o, nkb, kb0, pt)
                            finish_head(0, h, ps_o, dacc, at_tiles)

                        sc0(0)
                        sc0(1)
                        sc0(2)
                        pv0(0)
                        sc0(3)
                        pv0(1)
                        pv0(2)
                        pv0(3)
                    else:
                        for h in range(QH_PER_CORE):
                            qs = qt_sb[:, h, t * 512 : (t + 1) * 512]
                            ps_o = pbp.tile([128, 512], dt.float32,
                                            tag="attnT", bufs=1)
                            dacc = pb.tile([128, 512], dt.bfloat16,
                                           tag="dacc", bufs=4)
                            for kb0 in range(0, nkb, 2):
                                pt = scores_pair(t, h, qs, kb0, dacc)
                                pv_pair(t, ps_o, nkb, kb0, pt)
                            finish_head(t, h, ps_o, dacc, at_tiles)
                            if prev_ats is not None:
                                oproj_group(t - 1, h, prev_ats)
                    prev_ats = at_tiles
                if prev_ats is not None:
                    for sbl in range(4):
                        oproj_group(NT - 1, sbl, prev_ats)
    nc.compile()
    return nc


def _prep_inputs(x, cos, sin, wq, wk, wv, wo):
    x = np.asarray(x, dtype=np.float32)
    cos = np.asarray(cos, dtype=np.float32)
    sin = np.asarray(sin, dtype=np.float32)
    wq = np.asarray(wq, dtype=np.float32)
    wk = np.asarray(wk, dtype=np.float32)
    wv = np.asarray(wv, dtype=np.float32)
    wo = np.asarray(wo, dtype=np.float32)

    xt = np.ascontiguousarray(x[0].T).astype(bf16)  # [DIM, S]
    # cos/sin transposed and duplicated into both partition halves [128, S]
    csd = np.ascontiguousarray(np.tile(cos.T, (2, 1)).astype(bf16))
    snd = np.ascontiguousarray(np.tile(sin.T, (2, 1)).astype(bf16))
    # de-interleave perm: head dim pairs (2i, 2i+1) -> rows (i, 64+i)
    perm = np.concatenate([np.arange(0, HD, 2), np.arange(1, HD, 2)])

    # causal triangular mask for a diagonal 128x128 block: keep k <= q
    r = np.arange(128)[:, None]
    c = np.arange(128)[None, :]
    tri = np.ascontiguousarray((r <= c).astype(bf16))

    in_maps = []
    for i in range(N_CORES):
        wq_i = wq[DQ * i : DQ * (i + 1)]  # [512, DIM]
        wk_i = wk[HD * i : HD * (i + 1)]  # [128, DIM]
        wv_i = wv[HD * i : HD * (i + 1)]
        wq_p = wq_i.reshape(QH_PER_CORE, HD, DIM)[:, perm, :].reshape(DQ, DIM)
        wk_p = wk_i[perm, :]
        wt = np.concatenate([wq_p.T, wk_p.T, wv_i.T], axis=1).astype(bf16)
        wot = np.ascontiguousarray(wo[:, DQ * i : DQ * (i + 1)].T).astype(
            bf16
        )  # [512, DIM]
        in_maps.append(
            {
                "xt": xt,
                "wt": np.ascontiguousarray(wt),
                "wot": wot,
                "csd": csd,
                "snd": snd,
                "tri": tri,
            }
        )
    return in_maps


def _get_runner():
    global _RUNNER
    if _RUNNER is None:
        _RUNNER = _build()
    return _RUNNER


def kernel(x, cos, sin, wq, wk, wv, wo):
    from concourse.bass_utils import run_bass_kernel_spmd

    nc = _get_runner()
    in_maps = _prep_inputs(x, cos, sin, wq, wk, wv, wo)
    res = run_bass_kernel_spmd(nc, in_maps, list(range(N_CORES)))
    out = np.zeros((S, DIM), dtype=np.float32)
    for i in range(N_CORES):
        out += np.asarray(res.results[i]["out"], dtype=np.float32)
    return out[None].astype(np.float32)

